# revision 16
# baseline (speedup 1.0000x reference)
"""Trainium2 Bass kernel for gated-adapter attention (Llama-Adapter style).

Sharding: 2 data-parallel groups of 4 cores (batch dim); within a group the 32
heads are tensor-parallel (8 heads/core).  Each core computes QKV + RoPE +
causal flash-style attention (transposed scores) + gated adapter cross
attention for its heads, AllGathers the per-head attention output across its
group of 4, then computes a column shard of the wo projection.  The host
reassembles the full [2, 2048, 4096] output from the 8 per-core shards.

v2 changes vs baseline:
  - x is host-prepped into chunk-major layout; one DMA per 1MB chunk
    (8KB/partition lines) so LDWEIGHTS always overlaps the previous matmul.
  - adapter K/V projections precomputed on host (10 tokens, 0.05% of FLOPs).
  - weight loads batched 2 m-blocks per DMA with cross-pass prefetch.
  - RoPE reads the QKV psum directly (no intermediate SBUF copy).
  - attention software-pipelined: sc(i) runs `LEAD` chunks ahead of
    sums/pv(i) so the PE never waits on scalar exp.
  - phase 2 and phase 3 share one ExitStack (no pool barrier); wo weights
    prefetch during attention; gathered activations load in 2 big DMAs.
"""

import math
import os
import sys

import numpy as np

for _p in ("/opt/trn_rl_repo",):
    if os.path.isdir(_p) and _p not in sys.path:
        sys.path.insert(0, _p)

import ml_dtypes  # noqa: E402,F401

import concourse.bass as bass  # noqa: E402
import concourse.mybir as mybir  # noqa: E402
import concourse.tile as tile  # noqa: E402
from concourse import bacc  # noqa: E402
from concourse import bass_isa  # noqa: E402

FP16 = np.float16
F16 = mybir.dt.float16
F32 = mybir.dt.float32

DIM = 4096
S = 2048
B = 2
H = 32
HD = 128
ALEN = 10

NCORES = 8
CPG = 4          # cores per group (group = one batch element)
HPC = 8          # heads per core
OC = HPC * HD    # 1024 output dims per core for q/k/v and for wo columns

TCN = 16         # t-chunks of 128 tokens (adapter handled on host)
NM = DIM // 128  # 32 contraction chunks
WPM = 2          # m-blocks per weight DMA tile
QB = 4           # query blocks
QW = 512         # query block width
SCALE = 1.0 / math.sqrt(HD)
LEAD = 1         # sc chunk-groups emitted ahead of sums/pv

REPLICA_GROUPS = [[0, 1, 2, 3], [4, 5, 6, 7]]

Exp = mybir.ActivationFunctionType.Exp
Copy = mybir.ActivationFunctionType.Copy


def _alu():
    from concourse.alu_op_type import AluOpType
    return AluOpType


def build_graph():
    nc = bacc.Bacc(
        "TRN2",
        target_bir_lowering=False,
        debug=False,
        num_devices=NCORES,
    )

    # ---- external I/O ------------------------------------------------------
    # x chunk-major: [tch, d%128, m, t] so one chunk loads with 8KB lines
    xc = nc.dram_tensor("xc", [TCN, 128, NM, 128], F16, kind="ExternalInput")
    wqT = nc.dram_tensor("wqT", [DIM, OC], F16, kind="ExternalInput")
    wkT = nc.dram_tensor("wkT", [DIM, OC], F16, kind="ExternalInput")
    wvT = nc.dram_tensor("wvT", [DIM, OC], F16, kind="ExternalInput")
    woT = nc.dram_tensor("woT", [DIM, OC], F16, kind="ExternalInput")
    cosP = nc.dram_tensor("cosP", [S, 64], F16, kind="ExternalInput")
    sinP = nc.dram_tensor("sinP", [S, 64], F16, kind="ExternalInput")
    maskmul = nc.dram_tensor("maskmul", [QB, 4, 128, QW], F16, kind="ExternalInput")
    akH = nc.dram_tensor("akH", [128, HPC * 16], F16, kind="ExternalInput")
    avH = nc.dram_tensor("avH", [16, HPC * HD], F16, kind="ExternalInput")
    eye = nc.dram_tensor("eye", [128, 128], F16, kind="ExternalInput")
    out_ext = nc.dram_tensor("out", [S, OC], F16, kind="ExternalOutput")

    op = _alu()

    with tile.TileContext(nc) as tc:
        with (
            tc.tile_pool(name="persist", bufs=1) as persist,
            tc.tile_pool(name="dram", bufs=1, space="DRAM") as dpool,
        ):
            kT = persist.tile([128, HPC * S], F16, tag="kT")     # [d, h*t]
            vsb = persist.tile([128, TCN * OC], F16, tag="vsb")  # [t, tc*o]
            akT = persist.tile([128, HPC * 16], F16, tag="akT")  # [d, h*16]
            avg = persist.tile([16, HPC * HD], F16, tag="avg")   # [a, h*d]
            ones = persist.tile([128, 1], F16, tag="ones")
            negC = persist.tile([128, 1], F32, tag="negC")

            nc.gpsimd.memset(ones[:], 1.0)
            nc.gpsimd.memset(negC[:], -9.0)
            nc.sync.dma_start(akT[:], akH[:])
            nc.sync.dma_start(avg[:], avH[:])

            qT = persist.tile([128, HPC * S], F16, tag="qT")
            agin = [dpool.tile([OC, QW], F16, tag=f"agin{q}", name=f"agin{q}")
                    for q in range(QB)]
            agout = [dpool.tile([CPG * OC, QW], F16, tag=f"agout{q}",
                                name=f"agout{q}")
                     for q in range(QB)]

            # ================= Phase 1: QKV + RoPE + transposes ============
            NWT = NM // WPM  # 16 weight tiles per projection
            with (
                tc.tile_pool(name="p1c", bufs=1) as p1c,
                tc.tile_pool(name="wres", bufs=NWT + 2) as wpool,
                tc.tile_pool(name="xin", bufs=2) as xpool,
                tc.tile_pool(name="rot", bufs=2) as rpool,
                tc.tile_pool(name="ps1", bufs=2, space="PSUM") as pspool,
                tc.tile_pool(name="pstr", bufs=2, space="PSUM") as ptpool,
            ):
                eyesb = p1c.tile([128, 128], F16, tag="eyesb")
                cossb = p1c.tile([128, 16 * 64], F16, tag="cossb")
                sinsb = p1c.tile([128, 16 * 64], F16, tag="sinsb")
                nc.sync.dma_start(eyesb[:], eye[:])
                nc.sync.dma_start(
                    cossb[:].rearrange("p (c d) -> p c d", c=16),
                    cosP[:].rearrange("(c p) d -> p c d", p=128),
                )
                nc.sync.dma_start(
                    sinsb[:].rearrange("p (c d) -> p c d", c=16),
                    sinP[:].rearrange("(c p) d -> p c d", p=128),
                )
                first = True
                for proj, wsrc in (("q", wqT), ("k", wkT), ("v", wvT)):
                    wres = []
                    for j in range(NWT):
                        if first and j == 0:
                            # get chunk 0 of x in flight before the weights
                            x0 = xpool.tile([128, NM * 128], F16, tag="xin",
                                            name="xq0")
                            nc.sync.dma_start(
                                x0[:].rearrange("p (m t) -> p m t", m=NM),
                                xc[0])
                        wt = wpool.tile([128, WPM * 1024], F16, tag="wres",
                                        name=f"w{proj}{j}")
                        nc.sync.dma_start(
                            wt[:].rearrange("p (f o) -> p f o", f=WPM),
                            wsrc[j * WPM * 128:(j + 1) * WPM * 128, :]
                            .rearrange("(f p) o -> p f o", p=128),
                        )
                        wres.append(wt)
                    for tch in range(TCN):
                        if first and tch == 0:
                            xb = x0
                        else:
                            xb = xpool.tile([128, NM * 128], F16, tag="xin",
                                            name=f"x{proj}{tch}")
                            nc.sync.dma_start(
                                xb[:].rearrange("p (m t) -> p m t", m=NM),
                                xc[tch])
                        ps = pspool.tile([128, OC], F32, tag="ps1",
                                         name=f"ps{proj}{tch}")
                        for m in range(NM):
                            lhs = xb[:, m * 128:(m + 1) * 128]
                            wt = wres[m // WPM]
                            base = (m % WPM) * 1024
                            for half in range(2):
                                nc.tensor.matmul(
                                    ps[:, half * 512:(half + 1) * 512],
                                    lhsT=lhs,
                                    rhs=wt[:, base + half * 512:
                                           base + (half + 1) * 512],
                                    start=(m == 0),
                                    stop=(m == NM - 1),
                                )
                        first = False
                        if proj == "v":
                            nc.scalar.activation(
                                vsb[:, tch * OC:tch * OC + 512],
                                ps[:, 0:512], Copy)
                            nc.vector.tensor_copy(
                                vsb[:, tch * OC + 512:(tch + 1) * OC],
                                ps[:, 512:1024])
                            continue
                        # ---- RoPE (deinterleaved head_dim layout) --------
                        # reads psum halves directly
                        a3 = ps[:].rearrange(
                            "p (h d) -> p h d", h=HPC)[:, :, 0:64]
                        b3 = ps[:].rearrange(
                            "p (h d) -> p h d", h=HPC)[:, :, 64:128]
                        cos1 = cossb[:].rearrange(
                            "p (c o d) -> p c o d", c=16, o=1)[:, tch]
                        sin1 = sinsb[:].rearrange(
                            "p (c o d) -> p c o d", c=16, o=1)[:, tch]
                        cos3, _ = bass.broadcast_tensor_aps(cos1, a3)
                        sin3, _ = bass.broadcast_tensor_aps(sin1, a3)
                        rot = rpool.tile([128, OC], F16, tag="rot",
                                         name=f"rot{proj}{tch}")
                        ra = rot[:].rearrange(
                            "p (h d) -> p h d", h=HPC)[:, :, 0:64]
                        rb = rot[:].rearrange(
                            "p (h d) -> p h d", h=HPC)[:, :, 64:128]
                        t1 = rpool.tile([128, HPC * 64], F16, tag="rt1",
                                        name=f"rt1{proj}{tch}")
                        t13 = t1[:].rearrange("p (h d) -> p h d", h=HPC)
                        t2 = rpool.tile([128, HPC * 64], F16, tag="rt2",
                                        name=f"rt2{proj}{tch}")
                        t23 = t2[:].rearrange("p (h d) -> p h d", h=HPC)
                        nc.vector.tensor_tensor(t13, a3, cos3, op.mult)
                        nc.vector.tensor_tensor(t23, b3, sin3, op.mult)
                        nc.vector.tensor_tensor(ra, t13, t23, op.subtract)
                        nc.vector.tensor_tensor(t13, a3, sin3, op.mult)
                        nc.vector.tensor_tensor(t23, b3, cos3, op.mult)
                        nc.vector.tensor_tensor(rb, t13, t23, op.add)
                        # ---- transpose per head -> kT / qT ---------------
                        ptr = ptpool.tile([128, OC], F16, tag="pstr",
                                          name=f"ptr{proj}{tch}")
                        for h in range(HPC):
                            nc.tensor.transpose(
                                ptr[:, h * 128:(h + 1) * 128],
                                rot[:, h * 128:(h + 1) * 128],
                                eyesb[:],
                            )
                        dst = kT if proj == "k" else qT
                        nc.scalar.activation(
                            dst[:].rearrange(
                                "p (h t) -> p h t",
                                h=HPC)[:, :, tch * 128:(tch + 1) * 128],
                            ptr[:].rearrange("p (h d) -> p h d", h=HPC),
                            Copy,
                        )

            # ============ Phase 2+3: attention / AllGather / wo ============
            from contextlib import ExitStack
            _es = ExitStack()
            with _es:
                P = lambda **kw: _es.enter_context(tc.tile_pool(**kw))
                prpool = P(name="probs", bufs=4)
                mkpool = P(name="mask", bufs=2)

                def load_mask(qb):
                    mt = mkpool.tile([128, 4 * QW], F16, tag="mask",
                                     name=f"mm{qb}")
                    nc.sync.dma_start(
                        mt[:].rearrange("p (k t) -> p k t", k=4),
                        maskmul[qb].rearrange("k p t -> p k t"),
                    )
                    return mt
                smpool = P(name="small", bufs=1)
                rcpool = P(name="rec", bufs=1)
                bcpool = P(name="bcast", bufs=1)
                c3pool = P(name="c3p", bufs=3)
                arpool = P(name="arp", bufs=1)
                pscp = P(name="psc", bufs=3, space="PSUM")
                ppvp = P(name="ppv", bufs=2, space="PSUM")
                psmp = P(name="psums", bufs=2, space="PSUM")
                # phase-3 pools (shared ExitStack: no barrier, early prefetch)
                w2pool = P(name="w2", bufs=33)
                agpool = P(name="agsb", bufs=5)
                ostpool = P(name="ost", bufs=1)

                def load_w2(jh):
                    tiles = []
                    for m in range(NM):
                        wt = w2pool.tile([128, 512], F16, tag="w2",
                                         name=f"w2_{jh}_{m}")
                        nc.sync.dma_start(
                            wt[:], woT[m * 128:(m + 1) * 128,
                                       jh * 512:(jh + 1) * 512])
                        tiles.append(wt)
                    return tiles

                def attention_pair(qh, ql, mth, mtl):
                    """Process q-blocks qh and ql together: consecutive
                    matmuls share each stationary operand (kT chunk, ones,
                    vsb slice) so LDWEIGHTS always has a full matmul of
                    cover and the weight-settle latency stays hidden."""
                    kkh, kkl = (qh + 1) * 4, (ql + 1) * 4
                    for h in range(HPC):
                        qh_ap = qT[:, h * S + qh * QW: h * S + (qh + 1) * QW]
                        ql_ap = qT[:, h * S + ql * QW: h * S + (ql + 1) * QW]
                        sumh = psmp.tile([1, QW], F32, tag="sums",
                                         name=f"sumh{qh}_{h}")
                        suml = psmp.tile([1, QW], F32, tag="sums",
                                         name=f"suml{qh}_{h}")
                        pvh = ppvp.tile([128, QW], F32, tag="pv",
                                        name=f"pvh{qh}_{h}")
                        pvl = ppvp.tile([128, QW], F32, tag="pv",
                                        name=f"pvl{qh}_{h}")

                        def emit_sc(qb, kc, q_ap, mt):
                            sc = pscp.tile([128, QW], F32, tag="sc",
                                           name=f"sc{qb}_{h}_{kc}")
                            nc.tensor.matmul(
                                sc[:],
                                lhsT=kT[:, h * S + kc * 128:
                                        h * S + (kc + 1) * 128],
                                rhs=q_ap, start=True, stop=True)
                            pb = prpool.tile([128, QW], F16, tag="probs",
                                             name=f"pb{qb}_{h}_{kc}")
                            nc.scalar.activation(pb[:], sc[:], Exp,
                                                 bias=negC[:, 0:1],
                                                 scale=SCALE)
                            if kc >= qb * 4:
                                dk = kc - qb * 4
                                nc.vector.tensor_tensor(
                                    pb[:], pb[:],
                                    mt[:, dk * QW:(dk + 1) * QW], op.mult)
                            return pb

                        def emit_red(kc, pbh, pbl):
                            nc.tensor.matmul(
                                sumh[:], lhsT=ones[:, 0:1], rhs=pbh[:],
                                start=(kc == 0), stop=(kc == kkh - 1))
                            if pbl is not None:
                                nc.tensor.matmul(
                                    suml[:], lhsT=ones[:, 0:1],
                                    rhs=pbl[:],
                                    start=(kc == 0), stop=(kc == kkl - 1))
                            vs = vsb[:, kc * OC + h * HD:
                                     kc * OC + (h + 1) * HD]
                            nc.tensor.matmul(
                                pvh[:], lhsT=vs, rhs=pbh[:],
                                start=(kc == 0), stop=(kc == kkh - 1))
                            if pbl is not None:
                                nc.tensor.matmul(
                                    pvl[:], lhsT=vs, rhs=pbl[:],
                                    start=(kc == 0), stop=(kc == kkl - 1))

                        pend = []
                        for kc in range(kkh):
                            pbh = emit_sc(qh, kc, qh_ap, mth)
                            pbl = (emit_sc(ql, kc, ql_ap, mtl)
                                   if kc < kkl else None)
                            pend.append((kc, pbh, pbl))
                            if len(pend) > LEAD:
                                emit_red(*pend.pop(0))
                        while pend:
                            emit_red(*pend.pop(0))
                        # adapter (paired: akT, ones, avg each loaded once)
                        asch = pscp.tile([10, QW], F32, tag="sc",
                                         name=f"asch{qh}_{h}")
                        nc.tensor.matmul(
                            asch[:], lhsT=akT[:, h * 16:h * 16 + 10],
                            rhs=qh_ap, start=True, stop=True)
                        ascl = pscp.tile([10, QW], F32, tag="sc",
                                         name=f"ascl{qh}_{h}")
                        nc.tensor.matmul(
                            ascl[:], lhsT=akT[:, h * 16:h * 16 + 10],
                            rhs=ql_ap, start=True, stop=True)
                        apbh = smpool.tile([10, QW], F16, tag="aprobs",
                                           name=f"apbh{qh}_{h}")
                        nc.scalar.activation(apbh[:], asch[:], Exp,
                                             bias=negC[0:10, 0:1],
                                             scale=SCALE)
                        apbl = smpool.tile([10, QW], F16, tag="aprobs2",
                                           name=f"apbl{qh}_{h}")
                        nc.scalar.activation(apbl[:], ascl[:], Exp,
                                             bias=negC[0:10, 0:1],
                                             scale=SCALE)
                        arh = arpool.tile([10, QW], F32, tag="arh",
                                          name=f"arh{qh}_{h}")
                        nc.gpsimd.partition_all_reduce(
                            arh[:], apbh[:], 10, bass_isa.ReduceOp.add)
                        arl = arpool.tile([10, QW], F32, tag="arl",
                                          name=f"arl{qh}_{h}")
                        nc.gpsimd.partition_all_reduce(
                            arl[:], apbl[:], 10, bass_isa.ReduceOp.add)
                        apvh = pscp.tile([128, QW], F32, tag="sc",
                                         name=f"apvh{qh}_{h}")
                        nc.tensor.matmul(
                            apvh[:], lhsT=avg[0:10, h * HD:(h + 1) * HD],
                            rhs=apbh[:], start=True, stop=True)
                        apvl = pscp.tile([128, QW], F32, tag="sc",
                                         name=f"apvl{qh}_{h}")
                        nc.tensor.matmul(
                            apvl[:], lhsT=avg[0:10, h * HD:(h + 1) * HD],
                            rhs=apbl[:], start=True, stop=True)
                        # normalize + combine per q-block
                        for qb, pv, apv, sm, ar in ((qh, pvh, apvh, sumh, arh),
                                                    (ql, pvl, apvl, suml, arl)):
                            recM = rcpool.tile([1, QW], F32, tag="recM",
                                               name=f"rM{qb}_{h}")
                            nc.vector.reciprocal_approx_fast(
                                recM[:], sm[:])
                            recA = rcpool.tile([1, QW], F32, tag="recA",
                                               name=f"rA{qb}_{h}")
                            nc.vector.reciprocal_approx_fast(
                                recA[:], ar[0:1, :])
                            bcM = bcpool.tile([128, QW], F32, tag="bcM",
                                              name=f"bM{qb}_{h}")
                            nc.gpsimd.partition_broadcast(bcM[:], recM[:])
                            bcA = bcpool.tile([128, QW], F32, tag="bcA",
                                              name=f"bA{qb}_{h}")
                            nc.gpsimd.partition_broadcast(bcA[:], recA[:])
                            nc.vector.tensor_tensor(pv[:], pv[:], bcM[:],
                                                    op.mult)
                            c2 = bcpool.tile([128, QW], F32, tag="c2",
                                             name=f"c2{qb}_{h}")
                            nc.vector.tensor_tensor(c2[:], apv[:], bcA[:],
                                                    op.mult)
                            c3 = c3pool.tile([128, QW], F16, tag="c3",
                                             name=f"c3{qb}_{h}")
                            nc.vector.tensor_tensor(c3[:], pv[:], c2[:],
                                                    op.add)
                            nc.sync.dma_start(
                                agin[qb][h * 128:(h + 1) * 128, :], c3[:])
                    for qb in (qh, ql):
                        nc.gpsimd.collective_compute(
                            "AllGather",
                            op.bypass,
                            replica_groups=REPLICA_GROUPS,
                            ins=[agin[qb][:].opt()],
                            outs=[agout[qb][:].opt()],
                        )

                mth = load_mask(3)
                mtl = load_mask(2)
                # prefetch jh=0 wo weights during attention (after masks)
                w2t0 = load_w2(0)
                attention_pair(3, 2, mth, mtl)
                mth = load_mask(1)
                mtl = load_mask(0)
                attention_pair(1, 0, mth, mtl)

                # ---- wo projection -----------------------------------
                def wo_chunk(jh, w2t, qb):
                    ags = []
                    for quar in range(4):
                        a = agpool.tile([128, 8 * QW], F16, tag="agsb",
                                        name=f"ag{jh}_{qb}_{quar}")
                        nc.sync.dma_start(
                            a[:].rearrange("p (m t) -> p m t", m=8),
                            agout[qb][quar * 1024:(quar + 1) * 1024, :]
                            .rearrange("(m p) t -> p m t", p=128),
                        )
                        ags.append(a)
                    for tsub in range(4):
                        ps = ppvp.tile([128, 512], F32, tag="pv",
                                       name=f"pwo{jh}{qb}{tsub}")
                        for i in range(NM):
                            a = ags[i // 8]
                            base = (i % 8) * QW
                            nc.tensor.matmul(
                                ps[:],
                                lhsT=a[:, base + tsub * 128:
                                       base + (tsub + 1) * 128],
                                rhs=w2t[i][:],
                                start=(i == 0), stop=(i == NM - 1),
                            )
                        st = ostpool.tile([128, 512], F16, tag="ost",
                                          name=f"st{jh}{qb}{tsub}")
                        nc.scalar.activation(st[:], ps[:], Copy)
                        r0 = qb * QW + tsub * 128
                        nc.sync.dma_start(
                            out_ext[r0:r0 + 128,
                                    jh * 512:(jh + 1) * 512], st[:])

                for jh in range(2):
                    w2t = w2t0 if jh == 0 else load_w2(1)
                    for qb in range(QB - 1, -1, -1):
                        wo_chunk(jh, w2t, qb)

    nc.compile()
    return nc


# ---------------------------------------------------------------------------
# host-side input prep + execution
# ---------------------------------------------------------------------------

_DEINT = np.concatenate([np.arange(0, 128, 2), np.arange(1, 128, 2)])


def _prep_inputs(x, adapter, wq, wk, wv, wo, gate, freqs_cos, freqs_sin, mask):
    """Build the per-core input maps."""
    perm = np.concatenate([h * HD + _DEINT for h in range(H)])  # deinterleave
    wqp = wq[perm, :]  # permute output dims of wq/wk for rope layout
    wkp = wk[perm, :]

    # adapter K/V on host (10 tokens): ak/av [ALEN, DIM].  ak's output dims
    # must use the same deinterleave permutation as wq/wk so q.ak matches.
    ak = (adapter[0].astype(np.float32) @ wk.astype(np.float32).T)[:, perm]
    av = (adapter[0].astype(np.float32) @ wv.astype(np.float32).T)

    mm = np.empty((QB, 4, 128, QW), FP16)
    for qb in range(QB):
        q0 = qb * QW
        for dk in range(4):
            k0 = q0 + dk * 128
            mm[qb, dk] = np.exp(
                mask[0, 0, q0:q0 + QW, k0:k0 + 128]).T.astype(FP16)

    in_maps = []
    for c in range(NCORES):
        g, ci = divmod(c, CPG)
        osl = slice(ci * OC, (ci + 1) * OC)
        # x chunk-major: [tch, d%128, m, t]
        xg = x[g].T.astype(FP16)  # [DIM, S]
        xcm = np.ascontiguousarray(
            xg.reshape(NM, 128, TCN, 128).transpose(2, 1, 0, 3))
        # adapter tiles for this core's heads
        akc = np.zeros((128, HPC * 16), FP16)
        avc = np.zeros((16, HPC * HD), FP16)
        for h in range(HPC):
            gh = ci * HPC + h
            akc[:, h * 16:h * 16 + ALEN] = (
                ak[:, gh * HD:(gh + 1) * HD].T.astype(FP16))
            avc[0:ALEN, h * HD:(h + 1) * HD] = (
                av[:, gh * HD:(gh + 1) * HD]
                * gate[0, gh, 0, 0]).astype(FP16)
        in_maps.append({
            "xc": xcm,
            "wqT": np.ascontiguousarray(wqp[osl].T).astype(FP16),
            "wkT": np.ascontiguousarray(wkp[osl].T).astype(FP16),
            "wvT": np.ascontiguousarray(wv[osl].T).astype(FP16),
            "woT": np.ascontiguousarray(wo[osl].T).astype(FP16),
            "cosP": freqs_cos.astype(FP16),
            "sinP": freqs_sin.astype(FP16),
            "maskmul": mm,
            "akH": akc,
            "avH": avc,
            "eye": np.eye(128, dtype=FP16),
        })
    return in_maps


_NC_CACHE = {}
TRACE = bool(int(os.environ.get("BASS_KERNEL_TRACE", "0")))
LAST_EXEC_NS = None
LAST_RESULTS = None


def kernel(x, adapter, wq, wk, wv, wo, gate, freqs_cos, freqs_sin, mask,
           start_pos=0, **_unused):
    global LAST_EXEC_NS, LAST_RESULTS
    from concourse.bass_utils import run_bass_kernel_spmd

    to_np = lambda a: np.asarray(a)
    x, adapter, wq, wk, wv, wo = map(to_np, (x, adapter, wq, wk, wv, wo))
    gate, freqs_cos, freqs_sin, mask = map(
        to_np, (gate, freqs_cos, freqs_sin, mask))

    if "nc" not in _NC_CACHE:
        _NC_CACHE["nc"] = build_graph()
    nc = _NC_CACHE["nc"]

    in_maps = _prep_inputs(x, adapter, wq, wk, wv, wo, gate,
                           freqs_cos, freqs_sin, mask)
    res = run_bass_kernel_spmd(
        nc, in_maps, core_ids=list(range(NCORES)), trace=TRACE)
    LAST_EXEC_NS = res.exec_time_ns
    LAST_RESULTS = res
    out = np.empty((B, S, DIM), np.float32)
    for c in range(NCORES):
        g, ci = divmod(c, CPG)
        out[g, :, ci * OC:(ci + 1) * OC] = res.results[c]["out"].astype(np.float32)
    return out


# revision 19
# speedup vs baseline: 1.1018x; 1.1018x over previous
"""Trainium2 Bass kernel for gated-adapter attention (Llama-Adapter style).

Sharding: 2 data-parallel groups of 4 cores (batch dim); within a group the 32
heads are tensor-parallel (8 heads/core).  Each core computes QKV + RoPE +
causal flash-style attention (transposed scores) + gated adapter cross
attention for its heads, AllGathers the per-head attention output across its
group of 4, then computes a column shard of the wo projection.  The host
reassembles the full [2, 2048, 4096] output from the 8 per-core shards.

v2 changes vs baseline:
  - x is host-prepped into chunk-major layout; one DMA per 1MB chunk
    (8KB/partition lines) so LDWEIGHTS always overlaps the previous matmul.
  - adapter K/V projections precomputed on host (10 tokens, 0.05% of FLOPs).
  - weight loads batched 2 m-blocks per DMA with cross-pass prefetch.
  - RoPE reads the QKV psum directly (no intermediate SBUF copy).
  - attention software-pipelined: sc(i) runs `LEAD` chunks ahead of
    sums/pv(i) so the PE never waits on scalar exp.
  - phase 2 and phase 3 share one ExitStack (no pool barrier); wo weights
    prefetch during attention; gathered activations load in 2 big DMAs.
"""

import math
import os
import sys

import numpy as np

for _p in ("/opt/trn_rl_repo",):
    if os.path.isdir(_p) and _p not in sys.path:
        sys.path.insert(0, _p)

import ml_dtypes  # noqa: E402,F401

import concourse.bass as bass  # noqa: E402
import concourse.mybir as mybir  # noqa: E402
import concourse.tile as tile  # noqa: E402
from concourse import bacc  # noqa: E402
from concourse import bass_isa  # noqa: E402

FP16 = np.float16
F16 = mybir.dt.float16
F32 = mybir.dt.float32

DIM = 4096
S = 2048
B = 2
H = 32
HD = 128
ALEN = 10

NCORES = 8
CPG = 4          # cores per group (group = one batch element)
HPC = 8          # heads per core
OC = HPC * HD    # 1024 output dims per core for q/k/v and for wo columns

TCN = 16         # t-chunks of 128 tokens (adapter handled on host)
NM = DIM // 128  # 32 contraction chunks
WPM = 2          # m-blocks per weight DMA tile
QB = 4           # query blocks
QW = 512         # query block width
SCALE = 1.0 / math.sqrt(HD)
LEAD = 2         # sc chunks emitted ahead of sums/pv

REPLICA_GROUPS = [[0, 1, 2, 3], [4, 5, 6, 7]]

Exp = mybir.ActivationFunctionType.Exp
Copy = mybir.ActivationFunctionType.Copy


def _alu():
    from concourse.alu_op_type import AluOpType
    return AluOpType


def build_graph():
    nc = bacc.Bacc(
        "TRN2",
        target_bir_lowering=False,
        debug=False,
        num_devices=NCORES,
    )

    # ---- external I/O ------------------------------------------------------
    # x chunk-major: [tch, d%128, m, t] so one chunk loads with 8KB lines
    xc = nc.dram_tensor("xc", [TCN, 128, NM, 128], F16, kind="ExternalInput")
    wqT = nc.dram_tensor("wqT", [DIM, OC], F16, kind="ExternalInput")
    wkT = nc.dram_tensor("wkT", [DIM, OC], F16, kind="ExternalInput")
    wvT = nc.dram_tensor("wvT", [DIM, OC], F16, kind="ExternalInput")
    woT = nc.dram_tensor("woT", [DIM, OC], F16, kind="ExternalInput")
    cosP = nc.dram_tensor("cosP", [S, 64], F16, kind="ExternalInput")
    sinP = nc.dram_tensor("sinP", [S, 64], F16, kind="ExternalInput")
    trid = nc.dram_tensor("trid", [128, 128], F16, kind="ExternalInput")
    akH = nc.dram_tensor("akH", [128, HPC * 16], F16, kind="ExternalInput")
    avH = nc.dram_tensor("avH", [16, HPC * HD], F16, kind="ExternalInput")
    eye = nc.dram_tensor("eye", [128, 128], F16, kind="ExternalInput")
    out_ext = nc.dram_tensor("out", [S, OC], F16, kind="ExternalOutput")

    op = _alu()

    with tile.TileContext(nc) as tc:
        with (
            tc.tile_pool(name="persist", bufs=1) as persist,
            tc.tile_pool(name="dram", bufs=1, space="DRAM") as dpool,
        ):
            kT = persist.tile([128, HPC * S], F16, tag="kT")     # [d, h*t]
            vsb = persist.tile([128, TCN * OC], F16, tag="vsb")  # [t, tc*o]
            akT = persist.tile([128, HPC * 16], F16, tag="akT")  # [d, h*16]
            avg = persist.tile([16, HPC * HD], F16, tag="avg")   # [a, h*d]
            ones = persist.tile([128, 1], F16, tag="ones")
            negC = persist.tile([128, 1], F32, tag="negC")

            nc.gpsimd.memset(ones[:], 1.0)
            nc.gpsimd.memset(negC[:], -9.0)
            nc.sync.dma_start(akT[:], akH[:])
            nc.sync.dma_start(avg[:], avH[:])

            qT = persist.tile([128, HPC * S], F16, tag="qT")
            agin = [dpool.tile([OC, QW], F16, tag=f"agin{q}", name=f"agin{q}")
                    for q in range(QB)]
            agout = [dpool.tile([CPG * OC, QW], F16, tag=f"agout{q}",
                                name=f"agout{q}")
                     for q in range(QB)]

            # ================= Phase 1: QKV + RoPE + transposes ============
            NWT = NM // WPM  # 16 weight tiles per projection
            with (
                tc.tile_pool(name="p1c", bufs=1) as p1c,
                tc.tile_pool(name="wres", bufs=NWT + 2) as wpool,
                tc.tile_pool(name="xin", bufs=2) as xpool,
                tc.tile_pool(name="rot", bufs=2) as rpool,
                tc.tile_pool(name="ps1", bufs=2, space="PSUM") as pspool,
                tc.tile_pool(name="pstr", bufs=2, space="PSUM") as ptpool,
            ):
                eyesb = p1c.tile([128, 128], F16, tag="eyesb")
                cossb = p1c.tile([128, 16 * 64], F16, tag="cossb")
                sinsb = p1c.tile([128, 16 * 64], F16, tag="sinsb")
                nc.sync.dma_start(eyesb[:], eye[:])
                nc.sync.dma_start(
                    cossb[:].rearrange("p (c d) -> p c d", c=16),
                    cosP[:].rearrange("(c p) d -> p c d", p=128),
                )
                nc.sync.dma_start(
                    sinsb[:].rearrange("p (c d) -> p c d", c=16),
                    sinP[:].rearrange("(c p) d -> p c d", p=128),
                )
                first = True
                for proj, wsrc in (("q", wqT), ("k", wkT), ("v", wvT)):
                    wres = []
                    for j in range(NWT):
                        if first and j == 0:
                            # get chunk 0 of x in flight before the weights
                            x0 = xpool.tile([128, NM * 128], F16, tag="xin",
                                            name="xq0")
                            nc.sync.dma_start(
                                x0[:].rearrange("p (m t) -> p m t", m=NM),
                                xc[0])
                        wt = wpool.tile([128, WPM * 1024], F16, tag="wres",
                                        name=f"w{proj}{j}")
                        nc.sync.dma_start(
                            wt[:].rearrange("p (f o) -> p f o", f=WPM),
                            wsrc[j * WPM * 128:(j + 1) * WPM * 128, :]
                            .rearrange("(f p) o -> p f o", p=128),
                        )
                        wres.append(wt)
                    for tch in range(TCN):
                        if first and tch == 0:
                            xb = x0
                        else:
                            xb = xpool.tile([128, NM * 128], F16, tag="xin",
                                            name=f"x{proj}{tch}")
                            nc.sync.dma_start(
                                xb[:].rearrange("p (m t) -> p m t", m=NM),
                                xc[tch])
                        ps = pspool.tile([128, OC], F32, tag="ps1",
                                         name=f"ps{proj}{tch}")
                        for m in range(NM):
                            lhs = xb[:, m * 128:(m + 1) * 128]
                            wt = wres[m // WPM]
                            base = (m % WPM) * 1024
                            for half in range(2):
                                nc.tensor.matmul(
                                    ps[:, half * 512:(half + 1) * 512],
                                    lhsT=lhs,
                                    rhs=wt[:, base + half * 512:
                                           base + (half + 1) * 512],
                                    start=(m == 0),
                                    stop=(m == NM - 1),
                                )
                        first = False
                        if proj == "v":
                            nc.scalar.activation(
                                vsb[:, tch * OC:tch * OC + 512],
                                ps[:, 0:512], Copy)
                            nc.vector.tensor_copy(
                                vsb[:, tch * OC + 512:(tch + 1) * OC],
                                ps[:, 512:1024])
                            continue
                        # ---- RoPE (deinterleaved head_dim layout) --------
                        # reads psum halves directly
                        a3 = ps[:].rearrange(
                            "p (h d) -> p h d", h=HPC)[:, :, 0:64]
                        b3 = ps[:].rearrange(
                            "p (h d) -> p h d", h=HPC)[:, :, 64:128]
                        cos1 = cossb[:].rearrange(
                            "p (c o d) -> p c o d", c=16, o=1)[:, tch]
                        sin1 = sinsb[:].rearrange(
                            "p (c o d) -> p c o d", c=16, o=1)[:, tch]
                        cos3, _ = bass.broadcast_tensor_aps(cos1, a3)
                        sin3, _ = bass.broadcast_tensor_aps(sin1, a3)
                        rot = rpool.tile([128, OC], F16, tag="rot",
                                         name=f"rot{proj}{tch}")
                        ra = rot[:].rearrange(
                            "p (h d) -> p h d", h=HPC)[:, :, 0:64]
                        rb = rot[:].rearrange(
                            "p (h d) -> p h d", h=HPC)[:, :, 64:128]
                        t1 = rpool.tile([128, HPC * 64], F16, tag="rt1",
                                        name=f"rt1{proj}{tch}")
                        t13 = t1[:].rearrange("p (h d) -> p h d", h=HPC)
                        t2 = rpool.tile([128, HPC * 64], F16, tag="rt2",
                                        name=f"rt2{proj}{tch}")
                        t23 = t2[:].rearrange("p (h d) -> p h d", h=HPC)
                        nc.vector.tensor_tensor(t13, a3, cos3, op.mult)
                        nc.vector.tensor_tensor(t23, b3, sin3, op.mult)
                        nc.vector.tensor_tensor(ra, t13, t23, op.subtract)
                        nc.vector.tensor_tensor(t13, a3, sin3, op.mult)
                        nc.vector.tensor_tensor(t23, b3, cos3, op.mult)
                        nc.vector.tensor_tensor(rb, t13, t23, op.add)
                        # ---- transpose per head -> kT / qT ---------------
                        ptr = ptpool.tile([128, OC], F16, tag="pstr",
                                          name=f"ptr{proj}{tch}")
                        for h in range(HPC):
                            nc.tensor.transpose(
                                ptr[:, h * 128:(h + 1) * 128],
                                rot[:, h * 128:(h + 1) * 128],
                                eyesb[:],
                            )
                        dst = kT if proj == "k" else qT
                        nc.scalar.activation(
                            dst[:].rearrange(
                                "p (h t) -> p h t",
                                h=HPC)[:, :, tch * 128:(tch + 1) * 128],
                            ptr[:].rearrange("p (h d) -> p h d", h=HPC),
                            Copy,
                        )

            # ============ Phase 2+3: attention / AllGather / wo ============
            from contextlib import ExitStack
            _es = ExitStack()
            with _es:
                P = lambda **kw: _es.enter_context(tc.tile_pool(**kw))
                prpool = P(name="probs", bufs=6)
                smpool = P(name="small", bufs=1)
                rcpool = P(name="rec", bufs=1)
                bcpool = P(name="bcast", bufs=1)
                c3pool = P(name="c3p", bufs=3)
                arpool = P(name="arp", bufs=1)
                pscp = P(name="psc", bufs=3, space="PSUM")
                ppvp = P(name="ppv", bufs=4, space="PSUM")
                psmp = P(name="psums", bufs=1, space="PSUM")
                # phase-3 pools (shared ExitStack: no barrier, early prefetch)
                w2pool = P(name="w2", bufs=33)
                agpool = P(name="agsb", bufs=5)
                ostpool = P(name="ost", bufs=1)

                def load_w2(jh):
                    tiles = []
                    for m in range(NM):
                        wt = w2pool.tile([128, 512], F16, tag="w2",
                                         name=f"w2_{jh}_{m}")
                        nc.sync.dma_start(
                            wt[:], woT[m * 128:(m + 1) * 128,
                                       jh * 512:(jh + 1) * 512])
                        tiles.append(wt)
                    return tiles

                def attention_block(qb, mtile):
                    kk = (qb + 1) * 4  # causal: k chunks 0..kk-1
                    for h in range(HPC):
                        q_ap = qT[:, h * S + qb * QW: h * S + (qb + 1) * QW]
                        sums = psmp.tile([1, QW], F32, tag="sums",
                                         name=f"sums{qb}_{h}")
                        pv = ppvp.tile([128, QW], F32, tag="pv",
                                       name=f"pv{qb}_{h}")
                        pbs = []

                        def emit_sc(kc):
                            # triangular trim: diagonal chunk dk skips its
                            # fully-masked first dk*128 columns
                            dk = kc - qb * 4
                            col0 = dk * 128 if dk > 0 else 0
                            sc = pscp.tile([128, QW], F32, tag="sc",
                                           name=f"sc{qb}_{h}_{kc}")
                            nc.tensor.matmul(
                                sc[:, col0:],
                                lhsT=kT[:, h * S + kc * 128:
                                        h * S + (kc + 1) * 128],
                                rhs=q_ap[:, col0:],
                                start=True, stop=True,
                            )
                            pb = prpool.tile([128, QW], F16, tag="probs",
                                             name=f"pb{qb}_{h}_{kc}")
                            nc.scalar.activation(pb[:, col0:], sc[:, col0:],
                                                 Exp, bias=negC[:, 0:1],
                                                 scale=SCALE)
                            if dk >= 0:
                                nc.vector.tensor_tensor(
                                    pb[:, col0:col0 + 128],
                                    pb[:, col0:col0 + 128],
                                    tri[:], op.mult)
                            pbs.append((pb, col0))

                        def emit_red(i):
                            pb, col0 = pbs[i]
                            nc.tensor.matmul(
                                sums[:, col0:], lhsT=ones[:, 0:1],
                                rhs=pb[:, col0:],
                                start=(i == 0), stop=(i == kk - 1),
                            )
                            nc.tensor.matmul(
                                pv[:, col0:],
                                lhsT=vsb[:, i * OC + h * HD:
                                         i * OC + (h + 1) * HD],
                                rhs=pb[:, col0:],
                                start=(i == 0), stop=(i == kk - 1),
                            )

                        for kc in range(kk):
                            emit_sc(kc)
                            if kc >= LEAD:
                                emit_red(kc - LEAD)
                        for i in range(kk - LEAD, kk):
                            emit_red(i)
                        # adapter
                        asc = pscp.tile([10, QW], F32, tag="sc",
                                        name=f"asc{qb}_{h}")
                        nc.tensor.matmul(
                            asc[:], lhsT=akT[:, h * 16:h * 16 + 10],
                            rhs=q_ap, start=True, stop=True)
                        apb = smpool.tile([10, QW], F16, tag="aprobs",
                                          name=f"apb{qb}_{h}")
                        nc.scalar.activation(apb[:], asc[:], Exp,
                                             bias=negC[0:10, 0:1],
                                             scale=SCALE)
                        arh = arpool.tile([10, QW], F32, tag="arh",
                                          name=f"ar{qb}_{h}")
                        nc.gpsimd.partition_all_reduce(
                            arh[:], apb[:], 10, bass_isa.ReduceOp.add)
                        apv = ppvp.tile([128, QW], F32, tag="pv",
                                        name=f"apv{qb}_{h}")
                        nc.tensor.matmul(
                            apv[:], lhsT=avg[0:10, h * HD:(h + 1) * HD],
                            rhs=apb[:], start=True, stop=True)
                        # normalize + combine
                        recM = rcpool.tile([1, QW], F32, tag="recM",
                                           name=f"rM{qb}_{h}")
                        nc.vector.reciprocal_approx_fast(recM[:], sums[:])
                        recA = rcpool.tile([1, QW], F32, tag="recA",
                                           name=f"rA{qb}_{h}")
                        nc.vector.reciprocal_approx_fast(recA[:], arh[0:1, :])
                        bcM = bcpool.tile([128, QW], F32, tag="bcM",
                                          name=f"bM{qb}_{h}")
                        nc.gpsimd.partition_broadcast(bcM[:], recM[:])
                        bcA = bcpool.tile([128, QW], F32, tag="bcA",
                                          name=f"bA{qb}_{h}")
                        nc.gpsimd.partition_broadcast(bcA[:], recA[:])
                        nc.vector.tensor_tensor(pv[:], pv[:], bcM[:],
                                                op.mult)
                        c2 = bcpool.tile([128, QW], F32, tag="c2",
                                         name=f"c2{qb}_{h}")
                        nc.vector.tensor_tensor(c2[:], apv[:], bcA[:],
                                                op.mult)
                        c3 = c3pool.tile([128, QW], F16, tag="c3",
                                         name=f"c3{qb}_{h}")
                        nc.vector.tensor_tensor(c3[:], pv[:], c2[:],
                                                op.add)
                        nc.sync.dma_start(
                            agin[qb][h * 128:(h + 1) * 128, :], c3[:])
                    nc.gpsimd.collective_compute(
                        "AllGather",
                        op.bypass,
                        replica_groups=REPLICA_GROUPS,
                        ins=[agin[qb][:].opt()],
                        outs=[agout[qb][:].opt()],
                    )

                tri = smpool.tile([128, 128], F16, tag="tri")
                nc.sync.dma_start(tri[:], trid[:])
                # prefetch jh=0 wo weights during attention
                w2t0 = load_w2(0)
                for qb in range(QB - 1, -1, -1):
                    attention_block(qb, None)

                # ---- wo projection -----------------------------------
                def wo_chunk(jh, w2t, qb):
                    ags = []
                    for quar in range(4):
                        a = agpool.tile([128, 8 * QW], F16, tag="agsb",
                                        name=f"ag{jh}_{qb}_{quar}")
                        nc.sync.dma_start(
                            a[:].rearrange("p (m t) -> p m t", m=8),
                            agout[qb][quar * 1024:(quar + 1) * 1024, :]
                            .rearrange("(m p) t -> p m t", p=128),
                        )
                        ags.append(a)
                    for tsub in range(4):
                        ps = ppvp.tile([128, 512], F32, tag="pv",
                                       name=f"pwo{jh}{qb}{tsub}")
                        for i in range(NM):
                            a = ags[i // 8]
                            base = (i % 8) * QW
                            nc.tensor.matmul(
                                ps[:],
                                lhsT=a[:, base + tsub * 128:
                                       base + (tsub + 1) * 128],
                                rhs=w2t[i][:],
                                start=(i == 0), stop=(i == NM - 1),
                            )
                        st = ostpool.tile([128, 512], F16, tag="ost",
                                          name=f"st{jh}{qb}{tsub}")
                        nc.scalar.activation(st[:], ps[:], Copy)
                        r0 = qb * QW + tsub * 128
                        nc.sync.dma_start(
                            out_ext[r0:r0 + 128,
                                    jh * 512:(jh + 1) * 512], st[:])

                for jh in range(2):
                    w2t = w2t0 if jh == 0 else load_w2(1)
                    for qb in range(QB - 1, -1, -1):
                        wo_chunk(jh, w2t, qb)

    nc.compile()
    return nc


# ---------------------------------------------------------------------------
# host-side input prep + execution
# ---------------------------------------------------------------------------

_DEINT = np.concatenate([np.arange(0, 128, 2), np.arange(1, 128, 2)])


def _prep_inputs(x, adapter, wq, wk, wv, wo, gate, freqs_cos, freqs_sin, mask):
    """Build the per-core input maps."""
    perm = np.concatenate([h * HD + _DEINT for h in range(H)])  # deinterleave
    wqp = wq[perm, :]  # permute output dims of wq/wk for rope layout
    wkp = wk[perm, :]

    # adapter K/V on host (10 tokens): ak/av [ALEN, DIM].  ak's output dims
    # must use the same deinterleave permutation as wq/wk so q.ak matches.
    ak = (adapter[0].astype(np.float32) @ wk.astype(np.float32).T)[:, perm]
    av = (adapter[0].astype(np.float32) @ wv.astype(np.float32).T)

    tri = np.triu(np.ones((128, 128), FP16))

    in_maps = []
    for c in range(NCORES):
        g, ci = divmod(c, CPG)
        osl = slice(ci * OC, (ci + 1) * OC)
        # x chunk-major: [tch, d%128, m, t]
        xg = x[g].T.astype(FP16)  # [DIM, S]
        xcm = np.ascontiguousarray(
            xg.reshape(NM, 128, TCN, 128).transpose(2, 1, 0, 3))
        # adapter tiles for this core's heads
        akc = np.zeros((128, HPC * 16), FP16)
        avc = np.zeros((16, HPC * HD), FP16)
        for h in range(HPC):
            gh = ci * HPC + h
            akc[:, h * 16:h * 16 + ALEN] = (
                ak[:, gh * HD:(gh + 1) * HD].T.astype(FP16))
            avc[0:ALEN, h * HD:(h + 1) * HD] = (
                av[:, gh * HD:(gh + 1) * HD]
                * gate[0, gh, 0, 0]).astype(FP16)
        in_maps.append({
            "xc": xcm,
            "wqT": np.ascontiguousarray(wqp[osl].T).astype(FP16),
            "wkT": np.ascontiguousarray(wkp[osl].T).astype(FP16),
            "wvT": np.ascontiguousarray(wv[osl].T).astype(FP16),
            "woT": np.ascontiguousarray(wo[osl].T).astype(FP16),
            "cosP": freqs_cos.astype(FP16),
            "sinP": freqs_sin.astype(FP16),
            "trid": tri,
            "akH": akc,
            "avH": avc,
            "eye": np.eye(128, dtype=FP16),
        })
    return in_maps


_NC_CACHE = {}
TRACE = bool(int(os.environ.get("BASS_KERNEL_TRACE", "0")))
LAST_EXEC_NS = None
LAST_RESULTS = None


def kernel(x, adapter, wq, wk, wv, wo, gate, freqs_cos, freqs_sin, mask,
           start_pos=0, **_unused):
    global LAST_EXEC_NS, LAST_RESULTS
    from concourse.bass_utils import run_bass_kernel_spmd

    to_np = lambda a: np.asarray(a)
    x, adapter, wq, wk, wv, wo = map(to_np, (x, adapter, wq, wk, wv, wo))
    gate, freqs_cos, freqs_sin, mask = map(
        to_np, (gate, freqs_cos, freqs_sin, mask))

    if "nc" not in _NC_CACHE:
        _NC_CACHE["nc"] = build_graph()
    nc = _NC_CACHE["nc"]

    in_maps = _prep_inputs(x, adapter, wq, wk, wv, wo, gate,
                           freqs_cos, freqs_sin, mask)
    res = run_bass_kernel_spmd(
        nc, in_maps, core_ids=list(range(NCORES)), trace=TRACE)
    LAST_EXEC_NS = res.exec_time_ns
    LAST_RESULTS = res
    out = np.empty((B, S, DIM), np.float32)
    for c in range(NCORES):
        g, ci = divmod(c, CPG)
        out[g, :, ci * OC:(ci + 1) * OC] = res.results[c]["out"].astype(np.float32)
    return out
